# revision 1
# baseline (speedup 1.0000x reference)
"""Conv2D 3x3 stride-1 pad-1 (NCHW) as implicit GEMM on 8 NeuronCores.

Strategy: data-parallel over batch (32 imgs -> 4 per core). The input is
zero-padded on the host to (*, 128, 58, 58) so each image DMAs contiguously
into an SBUF tile [C=128, 58, 58] with input channels on partitions.
Weights are preprocessed host-side to [I=128, (kh kw o)] so each
(tap, ochunk) slice is a ready [K=128, M=128] stationary operand.
Output rows are processed in groups of 8 (moving free dim N = 8*56 = 448),
accumulating the 9 filter taps into one PSUM bank per row-group.

x (4,128,58,58) -> out (4,256,56,56) f32 per core; no collectives.
"""

import os
import sys

import numpy as np

if "/opt/trn_rl_repo" not in sys.path:
    sys.path.insert(0, "/opt/trn_rl_repo")

from concourse import bacc, bass, mybir  # noqa: E402
from concourse.bass_utils import run_bass_kernel_spmd  # noqa: E402
from concourse.tile import TileContext, add_dep_helper  # noqa: E402

N_FULL, CIN, H, W = 32, 128, 56, 56
COUT = 256
KH = KW = 3
NCORES = 8
NPER = N_FULL // NCORES  # 4 images per core
HP, WP = H + 2, W + 2  # 58 x 58 padded
ROWS = 8  # output rows per matmul group
NFREE = ROWS * W  # 448 moving free dim (<= 512 for 4-byte dtypes)
NGROUPS = H // ROWS  # 7
OCH = COUT // 128  # 2 output-channel chunks

# fp32r: full fp32 data streamed through the PE at bf16 rate (free dim >= 256).
MODE = os.environ.get("CONV_MM_MODE", "fp32r")

_CACHE = {}


def _build_conv(mode):
    f32 = mybir.dt.float32
    bf16 = mybir.dt.bfloat16
    if mode == "fp32":
        mm_dt, io_dt = f32, f32
    elif mode == "fp32r":
        mm_dt, io_dt = mybir.dt.float32r, f32
    elif mode in ("bf16", "split3"):
        mm_dt, io_dt = bf16, bf16
    else:
        raise ValueError(mode)

    # Bacc (not raw Bass): its compile pipeline legalizes sync waits --
    # TRN2 instructions carry at most one wait slot.
    nc = bacc.Bacc(None, target_bir_lowering=False)

    if mode == "split3":
        x_names = ["xh", "xl"]
        w_names = ["wh", "wl"]
        # (x_idx, w_idx) matmul passes: hh + hl + lh ~ full fp32 product
        terms = [(0, 0), (0, 1), (1, 0)]
    else:
        x_names = ["x"]
        w_names = ["wt"]
        terms = [(0, 0)]

    x_par = [
        nc.declare_dram_parameter(nm, [NPER, CIN, HP, WP], io_dt, isOutput=False)
        for nm in x_names
    ]
    w_par = [
        nc.declare_dram_parameter(nm, [CIN, KH * KW * COUT], io_dt, isOutput=False)
        for nm in w_names
    ]
    bias_par = nc.declare_dram_parameter("bias", [COUT], f32, isOutput=False)
    out_par = nc.declare_dram_parameter("out", [NPER, COUT, H, W], f32, isOutput=True)
    out_flat = out_par.rearrange("n o h w -> n o (h w)")

    def mmview(ap):
        return ap.bitcast(mm_dt) if mm_dt != io_dt else ap

    nmm_per_psum = KH * KW * len(terms)

    with TileContext(nc) as tc:
        with (
            tc.tile_pool(name="const", bufs=1) as cpool,
            tc.tile_pool(name="xpad", bufs=1) as xpool,
            tc.tile_pool(name="psum", bufs=8, space="PSUM") as ppool,
            tc.tile_pool(name="outp", bufs=4) as opool,
        ):
            # HAM pre-warm: junk matmuls gated only on a prologue memset run
            # during the initial DMA wait so the PE clock gate is at 8/8
            # (2.4 GHz) when the real stream starts. Results never consumed.
            jnk = cpool.tile([128, 512], f32, tag="jnk")
            nc.vector.memset(jnk[:], 1.0)
            jnk_mm = jnk if mm_dt == f32 else jnk.bitcast(mm_dt)
            ps_jnk = ppool.tile([128, NFREE], f32, tag="ps", name="ps")
            for _ in range(8):
                nc.tensor.matmul(
                    ps_jnk[:],
                    jnk_mm[:, 0:128],
                    jnk_mm[:, 0:NFREE],
                    start=True,
                    stop=True,
                )

            # Two padded-x buffers per input tensor (double buffering across
            # images); the zero borders come in with the host-padded DMA.
            xpads = []  # [buf][x_idx] -> tile
            for b in range(2):
                per_buf = []
                for xi in range(len(x_par)):
                    t = xpool.tile(
                        [CIN, HP, WP], mm_dt, tag=f"xpad{b}_{xi}", name="xpad"
                    )
                    per_buf.append(t)
                xpads.append(per_buf)

            # Head loads. Two constraints shape this: a single dma_start
            # tops out ~155 GB/s (vs ~358 GB/s HBM/core) so critical tensors
            # are split across two HW queues, and each issue costs ~0.7us
            # serially on the sync sequencer, so chunks are interleaved
            # x/w/x/w to overlap later issues with earlier transfers.
            # Deferred chunks (oc1 weight halves, image-0 rows 34+) ride
            # behind the first matmul.
            XSPLIT = 34  # padded rows [0,34) cover row-groups 0-3
            w_sb = []
            w3s = []
            for wi, wp in enumerate(w_par):
                t = cpool.tile([CIN, KH * KW * COUT], mm_dt, tag=f"w{wi}", name="w")
                w_sb.append(t)
                w3s.append(
                    (
                        t.rearrange("p (t o) -> p t o", t=KH * KW),
                        mmview(wp[:]).rearrange("p (t o) -> p t o", t=KH * KW),
                    )
                )
            for xi, xp in enumerate(x_par):
                nc.sync.dma_start(
                    out=xpads[0][xi][:, 0:17, :], in_=mmview(xp[0])[:, 0:17, :]
                )
            for t3, w3 in w3s:
                nc.sync.dma_start(out=t3[:, 0:5, 0:128], in_=w3[:, 0:5, 0:128])
            for xi, xp in enumerate(x_par):
                nc.sync.dma_start(
                    out=xpads[0][xi][:, 17:XSPLIT, :],
                    in_=mmview(xp[0])[:, 17:XSPLIT, :],
                )
            for t3, w3 in w3s:
                nc.sync.dma_start(out=t3[:, 5:9, 0:128], in_=w3[:, 5:9, 0:128])
            bias_sb = cpool.tile([128, OCH], f32, tag="bias")
            nc.sync.dma_start(
                out=bias_sb[:], in_=bias_par.rearrange("(a b) -> b a", b=128)
            )
            tail_dmas = []  # released once the first matmul has issued
            for xi, xp in enumerate(x_par):
                d = nc.sync.dma_start(
                    out=xpads[0][xi][:, XSPLIT:HP, :],
                    in_=mmview(xp[0])[:, XSPLIT:HP, :],
                )
                tail_dmas.append(d)
            for t3, w3 in w3s:
                d = nc.sync.dma_start(out=t3[:, :, 128:256], in_=w3[:, :, 128:256])
                tail_dmas.append(d)

            mm_first = None
            mm_oc1_first = None
            x1_dmas = []  # image-1 loads, deferred until the oc1 pass starts
            for n in range(NPER):
                bufs = xpads[n % 2]
                if n >= 1:
                    for xi, xp in enumerate(x_par):
                        # gpsimd queue: slot-reuse waits must not block the
                        # sync queue's output DMAs.
                        d = nc.gpsimd.dma_start(out=bufs[xi][:], in_=mmview(xp[n]))
                        if n == 1:
                            x1_dmas.append(d)
                for oc in range(OCH):
                    psums = [
                        ppool.tile([128, NFREE], f32, tag="ps", name="ps")
                        for _ in range(NGROUPS)
                    ]
                    i_mm = 0
                    for xi, wi in terms:
                        xt = bufs[xi]
                        for tap in range(KH * KW):
                            kh, kw = divmod(tap, KW)
                            lhsT = w_sb[wi][
                                :, tap * COUT + oc * 128 : tap * COUT + oc * 128 + 128
                            ]
                            for g in range(NGROUPS):
                                mm = nc.tensor.matmul(
                                    psums[g][:],
                                    lhsT,
                                    xt[
                                        :,
                                        g * ROWS + kh : g * ROWS + kh + ROWS,
                                        kw : kw + W,
                                    ],
                                    start=(i_mm == 0),
                                    stop=(i_mm == nmm_per_psum - 1),
                                )
                                if n == 0 and i_mm == 0 and g == 0:
                                    if oc == 0:
                                        mm_first = mm
                                    else:
                                        mm_oc1_first = mm
                            i_mm += 1
                    for g in range(NGROUPS):
                        ot = opool.tile([128, NFREE], f32, tag="ot", name="ot")
                        nc.vector.tensor_scalar_add(
                            out=ot[:], in0=psums[g][:], scalar1=bias_sb[:, oc : oc + 1]
                        )
                        nc.sync.dma_start(
                            out=out_flat[
                                n,
                                oc * 128 : (oc + 1) * 128,
                                g * NFREE : (g + 1) * NFREE,
                            ],
                            in_=ot[:],
                        )
            for d in tail_dmas:
                add_dep_helper(
                    d.ins, mm_first.ins, sync=True, reason="defer past first matmul"
                )
            for d in x1_dmas:
                add_dep_helper(
                    d.ins, mm_oc1_first.ins, sync=True, reason="defer image-1 load"
                )
    nc.compile()
    return nc


def _get_nc(mode):
    if mode not in _CACHE:
        _CACHE[mode] = _build_conv(mode)
    return _CACHE[mode]


# test-harness hooks: set TRACE=True before calling kernel() to capture an
# NTFF profile; LAST_RESULTS then holds the BassKernelResults.
TRACE = False
LAST_RESULTS = None


def kernel(x, weight, bias):
    global LAST_RESULTS
    mode = MODE
    x = np.ascontiguousarray(np.asarray(x), dtype=np.float32)
    w = np.ascontiguousarray(np.asarray(weight), dtype=np.float32)
    b = np.ascontiguousarray(np.asarray(bias), dtype=np.float32)
    xp = np.pad(x, ((0, 0), (0, 0), (1, 1), (1, 1)))
    # wt[i, (kh kw o)] = w[o, i, kh, kw]
    wt = np.ascontiguousarray(w.transpose(1, 2, 3, 0).reshape(CIN, KH * KW * COUT))

    if mode in ("fp32", "fp32r"):
        per_core = [
            {"x": xp[c * NPER : (c + 1) * NPER], "wt": wt, "bias": b}
            for c in range(NCORES)
        ]
    else:
        import ml_dtypes

        bfl = ml_dtypes.bfloat16
        if mode == "bf16":
            xh = xp.astype(bfl)
            wth = wt.astype(bfl)
            per_core = [
                {"x": xh[c * NPER : (c + 1) * NPER], "wt": wth, "bias": b}
                for c in range(NCORES)
            ]
        else:  # split3
            xh = xp.astype(bfl)
            xl = (xp - xh.astype(np.float32)).astype(bfl)
            wh = wt.astype(bfl)
            wl = (wt - wh.astype(np.float32)).astype(bfl)
            per_core = [
                {
                    "xh": xh[c * NPER : (c + 1) * NPER],
                    "xl": xl[c * NPER : (c + 1) * NPER],
                    "wh": wh,
                    "wl": wl,
                    "bias": b,
                }
                for c in range(NCORES)
            ]

    kwargs = {}
    if TRACE:
        kwargs = dict(trace=True, trace_cores=[0])
    res = run_bass_kernel_spmd(
        _get_nc(mode), per_core, core_ids=list(range(NCORES)), **kwargs
    )
    LAST_RESULTS = res
    return np.concatenate([r["out"] for r in res.results], axis=0)



# revision 7
# speedup vs baseline: 1.1569x; 1.1569x over previous
"""Conv2D 3x3 stride-1 pad-1 (NCHW) as implicit GEMM on 8 NeuronCores.

Strategy: data-parallel over batch (32 imgs -> 4 per core). The input is
zero-padded on the host to (*, 128, 58, 58) and cast to fp16 so each image
DMAs into an SBUF tile [C=128, 58, 58] with input channels on partitions.
Weights are preprocessed host-side to [I=128, oc_chunk, (kh kw) * 128] fp16
so each (oc, tap) slice is a ready [K=128, M=128] stationary operand.

Loop order is taps-INNER: for each output row-group (8 rows, free dim
8*56=448) the 9 filter taps accumulate back-to-back into one PSUM bank,
so finished groups stream out (bias-add on DVE + DMA) continuously
instead of bursting at the end of a pass. fp16 operands keep the per-MM
implicit LDWEIGHTS (~97ns) hidden under the 448-cycle matmul streaming;
fp32 weights would serialize at ~223ns/LDW and pace the whole kernel.

x (4,128,58,58) fp16 -> out (4,256,56,56) f32 per core; no collectives.
"""

import os
import sys

import numpy as np

if "/opt/trn_rl_repo" not in sys.path:
    sys.path.insert(0, "/opt/trn_rl_repo")

from concourse import bacc, bass, mybir  # noqa: E402
from concourse.bass_utils import run_bass_kernel_spmd  # noqa: E402
from concourse.tile import TileContext, add_dep_helper  # noqa: E402

N_FULL, CIN, H, W = 32, 128, 56, 56
COUT = 256
KH = KW = 3
NCORES = 8
NPER = N_FULL // NCORES  # 4 images per core
HP, WP = H + 2, W + 2  # 58 x 58 padded
ROWS = 8  # output rows per matmul group
NFREE = ROWS * W  # 448 moving free dim (<= 512 PSUM bank limit)
NGROUPS = H // ROWS  # 7
OCH = COUT // 128  # 2 output-channel chunks
NTAPS = KH * KW  # 9
NXBUF = 3  # x image buffers (triple buffer so loads run a full pass early)

MODE = os.environ.get("CONV_MM_MODE", "fp16")

_CACHE = {}


def _build_conv(mode):
    f32 = mybir.dt.float32
    io_dt = {
        "fp16": mybir.dt.float16,
        "bf16": mybir.dt.bfloat16,
        "fp32": f32,
        "fp32r": f32,
    }[mode]
    mm_dt = mybir.dt.float32r if mode == "fp32r" else io_dt

    # Bacc (not raw Bass): its compile pipeline legalizes sync waits --
    # TRN2 instructions carry at most one wait slot.
    nc = bacc.Bacc(None, target_bir_lowering=False)

    x_par = nc.declare_dram_parameter("x", [NPER, CIN, HP, WP], io_dt, isOutput=False)
    w_par = nc.declare_dram_parameter(
        "wt", [CIN, OCH, NTAPS * 128], io_dt, isOutput=False
    )
    bias_par = nc.declare_dram_parameter("bias", [COUT], f32, isOutput=False)
    out_par = nc.declare_dram_parameter("out", [NPER, COUT, H, W], f32, isOutput=True)
    out_flat = out_par.rearrange("n o h w -> n o (h w)")

    def mmv(ap):
        return ap.bitcast(mm_dt) if mm_dt != io_dt else ap

    with TileContext(nc) as tc:
        with (
            tc.tile_pool(name="const", bufs=1) as cpool,
            tc.tile_pool(name="xpad", bufs=1) as xpool,
            tc.tile_pool(name="psum", bufs=8, space="PSUM") as ppool,
            tc.tile_pool(name="outp", bufs=4) as opool,
        ):
            # HAM pre-warm: short junk matmuls gated only on a prologue
            # memset (scalar engine -- otherwise idle at the head) keep the
            # PE busy through the initial DMA wait so the clock gate is
            # released (2.4 GHz) near the start of the real stream. N=128
            # keeps them fine-grained so real matmuls slot in promptly.
            jnk = cpool.tile([128, 128], mm_dt, tag="jnk")
            nc.gpsimd.memset(jnk[:], 1.0)
            ps_jnk = ppool.tile([128, NFREE], f32, tag="ps", name="ps")
            for _ in range(12):
                nc.tensor.matmul(
                    ps_jnk[:, 0:128], jnk[:], jnk[:], start=True, stop=True
                )

            # x image buffers (zero borders come in with the host-padded DMA)
            xpads = [
                xpool.tile([CIN, HP, WP], mm_dt, tag=f"xpad{b}", name="xpad")
                for b in range(NXBUF)
            ]
            # weights: one tile per oc chunk, [CIN, (tap m)]
            w_sb = [
                cpool.tile([CIN, NTAPS * 128], mm_dt, tag=f"w{oc}", name="w")
                for oc in range(OCH)
            ]
            bias_sb = cpool.tile([128, OCH], f32, tag="bias")

            # Head loads. Constraints: a single dma_start tops out ~155 GB/s
            # and each issue costs ~0.6-0.7us serially on its queue's
            # sequencer, so the critical first tensors are spread across the
            # sync / gpsimd / vector queues and chunked so group-0 compute
            # can start while later rows are still in flight.
            XCH = [(0, 10), (10, 26), (26, 42), (42, 58)]  # image-0 row chunks
            nc.gpsimd.dma_start(
                out=w_sb[0][:, 0 : 5 * 128], in_=mmv(w_par)[:, 0, 0 : 5 * 128]
            )
            nc.scalar.dma_start(
                out=w_sb[0][:, 5 * 128 :], in_=mmv(w_par)[:, 0, 5 * 128 :]
            )
            for r0, r1 in XCH:
                nc.sync.dma_start(
                    out=xpads[0][:, r0:r1, :], in_=mmv(x_par[0])[:, r0:r1, :]
                )
            nc.gpsimd.dma_start(out=w_sb[1][:], in_=mmv(w_par)[:, 1, :])
            nc.scalar.dma_start(
                out=bias_sb[:], in_=bias_par.rearrange("(a b) -> b a", b=128)
            )

            # Image 1/2 land in fresh buffers (no WAR dep), so they are gated
            # behind the first real matmul to keep the head HBM window clean.
            # Image 3 reuses buffer 0: its dma_start MUST be emitted after
            # pass 0's matmuls in program order (inside the n loop below) so
            # the tile framework orders it write-after-read of pass 0 -- and
            # pass 0 reads image 0, not image 3.
            img_dmas = [
                nc.gpsimd.dma_start(out=xpads[n % NXBUF][:], in_=mmv(x_par[n]))
                for n in range(1, NXBUF)
            ]

            mm_first = None
            out_q = [nc.sync, nc.scalar]  # alternate output DMA queues
            qi = 0
            for n in range(NPER):
                xt = xpads[n % NXBUF]
                # load image n+2 into the buffer pass n-1 just released
                if NXBUF <= n + 2 < NPER:
                    nc.gpsimd.dma_start(
                        out=xpads[(n + 2) % NXBUF][:], in_=mmv(x_par[n + 2])
                    )
                for oc in range(OCH):
                    for g in range(NGROUPS):
                        ps = ppool.tile([128, NFREE], f32, tag="ps", name="ps")
                        for tap in range(NTAPS):
                            kh, kw = divmod(tap, KW)
                            mm = nc.tensor.matmul(
                                ps[:],
                                w_sb[oc][:, tap * 128 : (tap + 1) * 128],
                                xt[:, g * ROWS + kh : g * ROWS + kh + ROWS, kw : kw + W],
                                start=(tap == 0),
                                stop=(tap == NTAPS - 1),
                            )
                            if mm_first is None:
                                mm_first = mm
                        ot = opool.tile([128, NFREE], f32, tag="ot", name="ot")
                        nc.vector.tensor_scalar_add(
                            out=ot[:], in0=ps[:], scalar1=bias_sb[:, oc : oc + 1]
                        )
                        dst = out_flat[
                            n, oc * 128 : (oc + 1) * 128, g * NFREE : (g + 1) * NFREE
                        ]
                        last = n == NPER - 1 and oc == OCH - 1 and g == NGROUPS - 1
                        if last:
                            # split the final tile across both queues to
                            # halve the drain tail
                            hf = NFREE // 2
                            out_q[0].dma_start(out=dst[:, 0:hf], in_=ot[:, 0:hf])
                            out_q[1].dma_start(out=dst[:, hf:], in_=ot[:, hf:])
                        else:
                            out_q[qi].dma_start(out=dst, in_=ot[:])
                            qi ^= 1
            for d in img_dmas:
                add_dep_helper(
                    d.ins, mm_first.ins, sync=True, reason="defer past first matmul"
                )
    nc.compile()
    return nc


def _get_nc(mode):
    if mode not in _CACHE:
        _CACHE[mode] = _build_conv(mode)
    return _CACHE[mode]


# test-harness hooks: set TRACE=True before calling kernel() to capture an
# NTFF profile; LAST_RESULTS then holds the BassKernelResults.
TRACE = False
LAST_RESULTS = None


def kernel(x, weight, bias):
    global LAST_RESULTS
    mode = MODE
    x = np.ascontiguousarray(np.asarray(x), dtype=np.float32)
    w = np.ascontiguousarray(np.asarray(weight), dtype=np.float32)
    b = np.ascontiguousarray(np.asarray(bias), dtype=np.float32)
    xp = np.pad(x, ((0, 0), (0, 0), (1, 1), (1, 1)))
    # wt[i, oc, (kh kw m)] = w[oc*128 + m, i, kh, kw]
    wt = np.ascontiguousarray(
        w.transpose(1, 2, 3, 0)
        .reshape(CIN, NTAPS, OCH, 128)
        .transpose(0, 2, 1, 3)
        .reshape(CIN, OCH, NTAPS * 128)
    )

    if mode in ("fp32", "fp32r"):
        xc, wc = xp, wt
    elif mode == "fp16":
        xc, wc = xp.astype(np.float16), wt.astype(np.float16)
    else:  # bf16
        import ml_dtypes

        xc = xp.astype(ml_dtypes.bfloat16)
        wc = wt.astype(ml_dtypes.bfloat16)

    per_core = [
        {"x": xc[c * NPER : (c + 1) * NPER], "wt": wc, "bias": b}
        for c in range(NCORES)
    ]

    kwargs = {}
    if TRACE:
        kwargs = dict(trace=True, trace_cores=[0])
    res = run_bass_kernel_spmd(
        _get_nc(mode), per_core, core_ids=list(range(NCORES)), **kwargs
    )
    LAST_RESULTS = res
    return np.concatenate([r["out"] for r in res.results], axis=0)
